# revision 8
# baseline (speedup 1.0000x reference)
"""Trainium2 Bass kernel v2 for nn_DeepONetCfCDecoder.

Strategy (8 cores, data-parallel over queries, time-banded as baseline):
  * Phase T (batched over all Q=TPC*128 queries, T-layout [rows, Q]):
    features built via PE broadcasts + gpsimd partition_broadcast, trunk MLP,
    q-projection WITHOUT LayerNorm (LN1 folded into attention: per-query
    rstd as Exp scale, rank-2 correction matmul with rows (m, 1/rstd) against
    host-precomputed (-ku, kb) token rows), LN1 stats via X^T X diag trick,
    tb (+bias via DVE add), per-tile one-hot/final-coef precomputes.
  * Phase P1: K^T/V interleaved table (ctab) as baseline.
  * Phase A1 (exp act-table): per tile scores (4 mm) + mask mm + corr mm from
    the dynamic ctab window (no slab copy), Exp(scale=rstd), transpose, ctx,
    LN2 via bn_stats/bn_aggr + Newton rsqrt, cw1 -> z stored.
  * Phase A2 (silu act-table): silu(z), cw2, residual, bp, rank contraction
    with bias folded via ttr initial values, single batched output DMA.
  Activation-table loads: 3 total (silu/sin set -> exp set -> silu set).
"""

import sys

sys.path.insert(0, "/opt/trn_rl_repo")

import numpy as np
import ml_dtypes

import concourse.bass as bass
import concourse.mybir as mybir
import concourse.tile as tile
import bass_rust as _bass_rust
from concourse.bass_utils import run_bass_kernel_spmd

BF16 = ml_dtypes.bfloat16
F32 = mybir.dt.float32
BF = mybir.dt.bfloat16
I32 = mybir.dt.int32
AF = mybir.ActivationFunctionType
ALU = mybir.AluOpType

N, K, T, D = 8192, 64, 512, 256
H, RANK, DTDIM, FH, L = 256, 256, 32, 8, 1.0
NCORES = 8
P = 128         # queries per tile
NEG = -30000.0  # additive mask value
MAGIC = float(1.5 * 2 ** 23)
TWO_PI = float(2 * np.pi)


def _pack(t_q, sensor_time, G):
    """Sort queries by bucket, chunk to cores, pack 128-query tiles."""
    idx = np.clip(np.searchsorted(sensor_time, t_q, side="right") - 1, 0, T - 1)
    order = np.argsort(idx, kind="stable")
    per_core = N // NCORES
    raw = []
    maxB = maxTPC = 0
    for i in range(NCORES):
        sel = order[i * per_core:(i + 1) * per_core]
        bidx = idx[sel]
        lo = int(bidx[0])
        Bc = int(bidx[-1]) - lo + 1
        tiles = []
        pos = 0
        while pos < len(sel):
            b0 = int(bidx[pos]) - lo
            s = b0 - (b0 % 2)
            take, g = [], []
            while pos < len(sel) and len(take) < P and int(bidx[pos]) - lo < s + G:
                take.append(sel[pos])
                g.append(int(bidx[pos]) - lo - s)
                pos += 1
            nreal = len(take)
            while len(take) < P:
                take.append(take[-1])
                g.append(g[-1])
            tiles.append([s, np.array(take), np.array(g, np.int64), nreal])
        raw.append((lo, Bc, tiles))
        maxB = max(maxB, Bc)
        maxTPC = max(maxTPC, len(tiles))
    B = max(maxB, G)
    B = (B + 7) // 8 * 8
    TPC = maxTPC
    cores = []
    for lo, Bc, tiles in raw:
        fixed = []
        for s, q, g, nr in tiles:
            s2 = min(s, B - G)
            fixed.append((s2, q, g + (s - s2), nr))
        while len(fixed) < TPC:
            fixed.append((0, fixed[-1][1], np.zeros(P, np.int64), 0))
        cores.append((lo, fixed))
    return cores, B, TPC, idx


def _build(B, TPC, G):
    B64 = B * 64
    Q = TPC * 128
    UW = (B // 2) * 512
    SL = G * 64            # slab keys per tile window
    NU = G // 2            # 2-bucket units per window
    NPC = SL // 512        # 512-wide score pieces
    assert SL % 512 == 0
    nc = bass.Bass()

    def inp(name, shape, dt=BF):
        return nc.declare_dram_parameter(name, list(shape), dt, isOutput=False)

    ht_d = inp("ht", [128, 2 * B64])
    wk_d = inp("wk", [128, 512])
    wv_d = inp("wv", [128, 512])
    trunkw_d = inp("trunkw", [72, 256])
    bq_d = inp("bqw", [128, 512])
    cw1_d = inp("cw1w", [128, 512])
    cw2_d = inp("cw2w", [128, 512])
    tow_d = inp("tow", [128, 1536])
    bpw_d = inp("bpw", [128, 1536])
    combt_d = inp("combt", [34, TPC * SL])  # expander rows | zeros | -ku,kb
    xyhl_d = inp("xyhl", [4, Q])           # rows: xh, xl, yh, yl
    dthl_d = inp("dthl", [2, Q])           # rows: dt_hi, dt_lo
    qrowb_d = inp("qrowb", [2, Q])         # bf16 rows: c, g
    moff_d = inp("moff", [1, TPC], I32)    # col m: 256*s
    repf_d = inp("repf", [128, 262], F32)  # cv rep | cs3 | cb3
    colc_d = inp("colc", [32, 4], F32)     # harm32, offs32, tpw, tpb
    emb3_d = inp("emb3", [3, 8])
    iotag_d = inp("iotag", [G, 1], F32)
    ppb_d = inp("ppb", [128, 4], F32)      # trunk_b c0,c1 | cb1 c0,c1
    ident_d = inp("ident", [128, 128])
    idf_d = inp("idf", [128, 128], F32)
    onesb_d = inp("onesb", [1, 128])
    repbf_d = inp("repbf", [128, 1536])    # to_b rep | bp_b_eff rep
    sel4_d = inp("sel4", [4, 32])
    ones2_d = inp("ones2", [2, 32])
    onescol_d = inp("onescol", [128, 1])
    out_d = nc.declare_dram_parameter("out", [128, TPC], F32, isOutput=True)

    qchunks = [(s, min(512, Q - s)) for s in range(0, Q, 512)]

    with tile.TileContext(nc) as tc:
        with (
            tc.tile_pool(name="const", bufs=1) as cp,
            tc.tile_pool(name="work", bufs=2) as wp,
            tc.tile_pool(name="slabp", bufs=2) as sp,
            tc.tile_pool(name="psA", bufs=2, space="PSUM") as ppA,
            tc.tile_pool(name="psB", bufs=3, space="PSUM") as ppB,
            tc.tile_pool(name="psT", bufs=2, space="PSUM") as ppT,
        ):
            # ------------- DMAs (ordered by first use) -------------
            onesb = cp.tile([1, 128], BF, tag="onesb")
            nc.sync.dma_start(onesb[:], onesb_d[:])
            wk = cp.tile([128, 512], BF, tag="wk")
            nc.sync.dma_start(wk[:], wk_d[:])
            wv = cp.tile([128, 512], BF, tag="wv")
            nc.sync.dma_start(wv[:], wv_d[:])
            ht = cp.tile([128, 2 * B64], BF, tag="ht")
            nc.sync.dma_start(ht[:], ht_d[:])
            repbf = cp.tile([128, 1536], BF, tag="repbf")
            nc.sync.dma_start(repbf[:], repbf_d[:])
            onescol = cp.tile([128, 1], BF, tag="onescol")
            nc.sync.dma_start(onescol[:], onescol_d[:])
            xyhl = cp.tile([4, Q], BF, tag="xyhl")
            nc.sync.dma_start(xyhl[:], xyhl_d[:])
            dthl = cp.tile([2, Q], BF, tag="dthl")
            nc.sync.dma_start(dthl[:], dthl_d[:])
            sel4 = cp.tile([4, 32], BF, tag="sel4")
            nc.sync.dma_start(sel4[:], sel4_d[:])
            ones2 = cp.tile([2, 32], BF, tag="ones2")
            nc.sync.dma_start(ones2[:], ones2_d[:])
            crow = cp.tile([1, Q], BF, tag="crow")
            nc.sync.dma_start(crow[:], qrowb_d[0:1, :])
            grow = cp.tile([1, Q], BF, tag="grow")
            nc.sync.dma_start(grow[:], qrowb_d[1:2, :])
            colc = cp.tile([32, 4], F32, tag="colc")
            nc.sync.dma_start(colc[:], colc_d[:])
            iotag = cp.tile([G, 1], F32, tag="iotag")
            nc.sync.dma_start(iotag[:], iotag_d[:])
            emb3 = cp.tile([3, 8], BF, tag="emb3")
            nc.sync.dma_start(emb3[:], emb3_d[:])
            ppb = cp.tile([128, 4], F32, tag="ppb")
            nc.sync.dma_start(ppb[:], ppb_d[:])
            ident = cp.tile([128, 128], BF, tag="ident")
            nc.sync.dma_start(ident[:], ident_d[:])
            idf = cp.tile([128, 128], F32, tag="idf")
            nc.sync.dma_start(idf[:], idf_d[:])
            trunkw = cp.tile([72, 256], BF, tag="trunkw")
            nc.sync.dma_start(trunkw[:], trunkw_d[:])
            moff = cp.tile([1, TPC], I32, tag="moff")
            nc.sync.dma_start(moff[:], moff_d[:])
            tow = cp.tile([128, 1536], BF, tag="tow")
            nc.sync.dma_start(tow[:], tow_d[:])
            repf = cp.tile([128, 262], F32, tag="repf")
            nc.sync.dma_start(repf[:], repf_d[:])
            bq = cp.tile([128, 512], BF, tag="bq")
            nc.sync.dma_start(bq[:], bq_d[:])
            combt = cp.tile([34, TPC * SL], BF, tag="combt")
            nc.sync.dma_start(combt[:], combt_d[:])
            cw1 = cp.tile([128, 512], BF, tag="cw1")
            nc.sync.dma_start(cw1[:], cw1_d[:])
            cw2 = cp.tile([128, 512], BF, tag="cw2")
            nc.sync.dma_start(cw2[:], cw2_d[:])
            bpw = cp.tile([128, 1536], BF, tag="bpw")
            nc.sync.dma_start(bpw[:], bpw_d[:])

            # ------------- persistent SBUF state -------------
            feat = cp.tile([72, Q], BF, tag="feat")
            ang = cp.tile([32, Q], F32, tag="ang")
            scr32 = cp.tile([32, Q], F32, tag="scr32")
            oh3T = cp.tile([3, Q], BF, tag="oh3T")
            comb = cp.tile([34, Q], BF, tag="comb")
            featTs = cp.tile([128, 2 * Q], BF, tag="featTs")
            featTs_v = featTs[:].rearrange("p (c q) -> p c q", q=Q)
            qT = cp.tile([128, 2 * Q], BF, tag="qT")
            qT_v = qT[:].rearrange("p (c q) -> p c q", q=Q)
            tb_all = cp.tile([128, 768 * TPC], BF, tag="tb_all")
            rstds = cp.tile([128, TPC], F32, tag="rstds")
            ohq_all = cp.tile([128, 3 * TPC], BF, tag="ohq_all")
            csoh_all = cp.tile([128, 3 * TPC], F32, tag="csoh_all")
            outb_all = cp.tile([128, TPC], F32, tag="outb_all")
            s3i = cp.tile([128, 3 * TPC], F32, tag="s3i")
            ctab = cp.tile([128, UW], BF, tag="ctab")
            ctab_v = ctab[:].rearrange("p (u blk) -> p u blk", blk=512)
            ctx_all = cp.tile([128, 256 * TPC], F32, tag="ctx_all")
            z_all = cp.tile([128, 2 * Q], BF, tag="z_all")
            z_v = z_all[:].rearrange("p (c q) -> p c q", q=Q)
            outbuf = cp.tile([128, TPC], F32, tag="outbuf")

            # ================= PHASE T =================
            # f32-exact broadcasts of x, y, dt on gpsimd
            # exact bf16-split PE broadcasts: xy32 rows (x*16 | y*16), dt rows
            for (s, w) in qchunks:
                pxy = ppA.tile([128, 512], F32, tag="A")
                nc.tensor.matmul(pxy[0:32, 0:w], sel4[:],
                                 xyhl[:, s:s + w], start=True, stop=True)
                nc.vector.tensor_scalar(
                    ang[:, s:s + w], pxy[0:32, 0:w], colc[:, 0:1],
                    colc[:, 1:2], ALU.mult, ALU.add)
                pdt = ppA.tile([128, 512], F32, tag="A")
                nc.tensor.matmul(pdt[0:32, 0:w], ones2[:],
                                 dthl[:, s:s + w], start=True, stop=True)
                nc.vector.tensor_scalar(
                    feat[32:64, s:s + w], pdt[0:32, 0:w], colc[:, 2:3],
                    colc[:, 3:4], ALU.mult, ALU.add)
            # fourier features: rows = sin(2pi*wrap(ang))
            nc.vector.tensor_scalar(
                scr32[:], ang[:], MAGIC, MAGIC, ALU.add, ALU.subtract)
            nc.vector.tensor_tensor(ang[:], ang[:], scr32[:], ALU.subtract)
            nc.scalar.activation(feat[0:32, :], ang[:], AF.Sin, scale=TWO_PI)

            # one-hots (bf16-exact PE broadcasts) + component embedding
            nc.vector.memset(comb[:], 0.0)
            for (s, w) in qchunks:
                p3 = ppA.tile([128, 512], F32, tag="A")
                nc.tensor.matmul(p3[0:3, 0:w], onesb[0:1, 0:3],
                                 crow[0:1, s:s + w], start=True, stop=True)
                nc.vector.tensor_scalar(
                    oh3T[:, s:s + w], p3[0:3, 0:w], iotag[0:3, 0:1], None,
                    ALU.is_equal)
                p12 = ppA.tile([128, 512], F32, tag="A")
                nc.tensor.matmul(p12[0:G, 0:w], onesb[0:1, 0:G],
                                 grow[0:1, s:s + w], start=True, stop=True)
                nc.vector.tensor_scalar(
                    comb[0:G, s:s + w], p12[0:G, 0:w], iotag[:, 0:1], None,
                    ALU.is_equal)
                p8 = ppA.tile([128, 512], F32, tag="A")
                nc.tensor.matmul(p8[0:8, 0:w], emb3[:], oh3T[:, s:s + w],
                                 start=True, stop=True)
                nc.vector.tensor_copy(feat[64:72, s:s + w], p8[0:8, 0:w])

            # trunk MLP (silu bias is per-partition in T-layout)
            for (s, w) in qchunks:
                for ich in range(2):
                    tp_ = ppA.tile([128, 512], F32, tag="A")
                    nc.tensor.matmul(
                        tp_[:, 0:w], trunkw[:, ich * 128:(ich + 1) * 128],
                        feat[:, s:s + w], start=True, stop=True)
                    nc.scalar.activation(
                        featTs_v[:, ich, s:s + w], tp_[:, 0:w], AF.Silu,
                        bias=ppb[:, ich:ich + 1])

            # q projection WITHOUT LayerNorm (scale 1/16 folded on host)
            for (s, w) in qchunks:
                for dch in range(2):
                    qp = ppA.tile([128, 512], F32, tag="A")
                    for hch in range(2):
                        nc.tensor.matmul(
                            qp[:, 0:w],
                            bq[:, (hch * 2 + dch) * 128:(hch * 2 + dch + 1) * 128],
                            featTs_v[:, hch, s:s + w],
                            start=(hch == 0), stop=(hch == 1))
                    nc.scalar.activation(qT_v[:, dch, s:s + w], qp[:, 0:w],
                                         AF.Copy)

            # per-tile: tb, LN1 stats (X^T X diag trick), final-coef precompute
            for m in range(TPC):
                msl = slice(m * 128, (m + 1) * 128)
                # tb = featTs^T @ to_w + to_b
                tbA = ppA.tile([128, 512], F32, tag="A")
                for hch in range(2):
                    nc.tensor.matmul(
                        tbA[:], featTs_v[:, hch, msl],
                        tow[:, hch * 768:hch * 768 + 512],
                        start=(hch == 0), stop=(hch == 1))
                nc.vector.tensor_tensor(
                    tb_all[:, m * 768:m * 768 + 512], tbA[:],
                    repbf[:, 0:512], ALU.add)
                tbB = ppB.tile([128, 256], F32, tag="B")
                for hch in range(2):
                    nc.tensor.matmul(
                        tbB[:], featTs_v[:, hch, msl],
                        tow[:, hch * 768 + 512:hch * 768 + 768],
                        start=(hch == 0), stop=(hch == 1))
                nc.vector.tensor_tensor(
                    tb_all[:, m * 768 + 512:m * 768 + 768], tbB[:],
                    repbf[:, 512:768], ALU.add)
                # s3 init: sum_r bp_bias*tb per comp
                scr_g = wp.tile([128, 256], F32, tag="scr_g")
                for comp in range(3):
                    nc.vector.scalar_tensor_tensor(
                        scr_g[:], tb_all[:, m * 768 + comp * 256:m * 768 + (comp + 1) * 256],
                        1.0, repbf[:, 768 + comp * 256:768 + (comp + 1) * 256],
                        ALU.mult, ALU.mult,
                        accum_out=s3i[:, m * 3 + comp:m * 3 + comp + 1])
                # LN1 stats: sum via ones rhs, sumsq via diag(X^T X)
                stp = ppB.tile([128, 256], F32, tag="B")
                for hch in range(2):
                    nc.tensor.matmul(
                        stp[:, 0:128], featTs_v[:, hch, msl],
                        featTs_v[:, hch, msl],
                        start=(hch == 0), stop=(hch == 1))
                for hch in range(2):
                    nc.tensor.matmul(
                        stp[:, 128:129], featTs_v[:, hch, msl], onescol[:],
                        start=(hch == 0), stop=(hch == 1))
                scrd = wp.tile([128, 128], F32, tag="scrd")
                sq = wp.tile([128, 1], F32, tag="sq")
                nc.vector.scalar_tensor_tensor(
                    scrd[:], stp[:, 0:128], 1.0, idf[:],
                    ALU.mult, ALU.mult, accum_out=sq[:])
                mcol = wp.tile([128, 1], F32, tag="mcol")
                nc.vector.tensor_scalar(mcol[:], stp[:, 128:129], 1.0 / 256,
                                        None, ALU.mult)
                t1 = wp.tile([128, 1], F32, tag="t1s")
                nc.vector.tensor_scalar(t1[:], sq[:], 1.0 / 256, 1e-5,
                                        ALU.mult, ALU.add)
                m2n = wp.tile([128, 1], F32, tag="m2n")
                nc.vector.tensor_scalar(m2n[:], mcol[:], mcol[:, 0:1], -1.0,
                                        ALU.mult, ALU.mult)
                ve = wp.tile([128, 1], F32, tag="ve")
                nc.vector.tensor_tensor(ve[:], t1[:], m2n[:], ALU.add)
                _newton(nc, wp, ve[:], rstds[:, m:m + 1], "n1")
                mi = wp.tile([128, 2], BF, tag="mi")
                nc.vector.tensor_copy(mi[:, 0:1], mcol[:])
                nc.vector.tensor_tensor(mi[:, 1:2], ve[:], rstds[:, m:m + 1],
                                        ALU.mult)
                mp_ = ppT.tile([128, SL], BF, tag="Tp")
                nc.tensor.transpose(mp_[0:2, 0:128], mi[:], ident[:])
                nc.vector.tensor_copy(comb[32:34, msl], mp_[0:2, 0:128])
                # one-hot per-query rows -> [q, 3] + final coefficients
                op2 = ppT.tile([128, SL], BF, tag="Tp")
                nc.tensor.transpose(op2[:, 0:3], oh3T[:, msl], ident[0:3, 0:3])
                nc.vector.tensor_copy(ohq_all[:, m * 3:(m + 1) * 3],
                                      op2[:, 0:3])
                nc.vector.tensor_tensor(
                    csoh_all[:, m * 3:(m + 1) * 3],
                    ohq_all[:, m * 3:(m + 1) * 3], repf[:, 256:259], ALU.mult)
                scr3 = wp.tile([128, 3], F32, tag="scr3")
                nc.vector.scalar_tensor_tensor(
                    scr3[:], ohq_all[:, m * 3:(m + 1) * 3], 1.0,
                    repf[:, 259:262], ALU.mult, ALU.mult,
                    accum_out=outb_all[:, m:m + 1])
                sc3b = wp.tile([128, 3], F32, tag="sc3b")
                s3icso = wp.tile([128, 1], F32, tag="s3icso")
                nc.vector.scalar_tensor_tensor(
                    sc3b[:], s3i[:, m * 3:(m + 1) * 3], 1.0,
                    csoh_all[:, m * 3:(m + 1) * 3], ALU.mult, ALU.mult,
                    accum_out=s3icso[:])
                nc.vector.tensor_tensor(
                    outb_all[:, m:m + 1], outb_all[:, m:m + 1], s3icso[:],
                    ALU.add)


            # ================= PHASE P1: K^T / V tables =================
            ei = 0

            def evict(dst, src):
                nonlocal ei
                if ei % 2 == 0:
                    nc.vector.tensor_copy(dst, src)
                else:
                    nc.scalar.activation(dst, src, AF.Copy)
                ei += 1

            for ch in range(2):
                for f0 in range(0, B64, 512):
                    ps = ppA.tile([128, 512], F32, tag="A")
                    for dch in range(2):
                        nc.tensor.matmul(
                            ps[:, 0:512],
                            wk[:, (dch * 2 + ch) * 128:(dch * 2 + ch + 1) * 128],
                            ht[:, dch * B64 + f0:dch * B64 + f0 + 512],
                            start=(dch == 0), stop=(dch == 1))
                    dst = ctab_v[:, f0 // 128:f0 // 128 + 4,
                                 ch * 128:(ch + 1) * 128]
                    psv = ps[:, 0:512].rearrange("p (u blk) -> p u blk", blk=128)
                    evict(dst, psv)
            for jp in range(B64 // 256):
                ps = ppA.tile([128, 512], F32, tag="A")
                for half in range(2):
                    js = 2 * jp + half
                    for dch in range(2):
                        nc.tensor.matmul(
                            ps[:, half * 256:(half + 1) * 256],
                            ht[:, dch * B64 + js * 128:dch * B64 + (js + 1) * 128],
                            wv[:, dch * 256:(dch + 1) * 256],
                            start=(dch == 0), stop=(dch == 1))
                dst = ctab_v[:, 2 * jp:2 * jp + 2, 256:512]
                psv2 = ps[:].rearrange("p (u blk) -> p u blk", blk=256)
                evict(dst, psv2)

            # ================= PHASE A1 (exp table) =================
            for m in range(TPC):
                msl = slice(m * 128, (m + 1) * 128)
                coff = nc.values_load(
                    moff[0:1, m:m + 1],
                    engines=[mybir.EngineType.DVE, mybir.EngineType.Activation],
                    min_val=0, max_val=UW - NU * 512,
                    skip_runtime_bounds_check=True)
                wsrc = ctab[:, bass.ds(coff, NU * 512)]
                slab = sp.tile([128, NU * 512], BF, tag="slab")
                c1 = 6 * (NU * 512) // 16
                nc.vector.tensor_copy(slab[:, 0:c1], wsrc[:, 0:c1])
                nc.scalar.activation(slab[:, c1:], wsrc[:, c1:], AF.Copy)
                win = slab[:].rearrange("p (u blk) -> p u blk", blk=512)

                expm = wp.tile([128, SL], BF, tag="expm")
                den2 = wp.tile([128, NPC], F32, tag="den2")
                for pc in range(NPC):
                    u0 = pc * 4
                    k0 = pc * 512
                    sc = ppA.tile([128, 512], F32, tag="A")
                    nc.tensor.matmul(sc[:], qT_v[:, 0, msl],
                                     win[:, u0:u0 + 4, 0:128],
                                     start=True, stop=False)
                    nc.tensor.matmul(sc[:], qT_v[:, 1, msl],
                                     win[:, u0:u0 + 4, 128:256],
                                     start=False, stop=False)
                    nc.tensor.matmul(
                        sc[:], comb[:, msl],
                        combt[:, m * SL + k0:m * SL + k0 + 512],
                        start=False, stop=True)
                    nc.scalar.activation(expm[:, k0:k0 + 512], sc[:], AF.Exp,
                                         scale=rstds[:, m:m + 1],
                                         accum_out=den2[:, pc:pc + 1])
                recip = wp.tile([128, 1], F32, tag="recip")
                nc.vector.tensor_tensor(recip[:], den2[:, 0:1], den2[:, 1:2],
                                        ALU.add)
                for pc in range(2, NPC):
                    nc.vector.tensor_tensor(recip[:], recip[:],
                                            den2[:, pc:pc + 1], ALU.add)
                nc.vector.reciprocal(recip[:], recip[:])

                tpC = ppT.tile([128, SL], BF, tag="Tp")
                for j in range(SL // 128):
                    nc.tensor.transpose(
                        tpC[:, j * 128:(j + 1) * 128],
                        expm[:, j * 128:(j + 1) * 128], ident[:])
                expT = wp.tile([128, SL], BF, tag="expT")
                nc.scalar.activation(expT[:], tpC[:], AF.Copy)

                ctx_ps = ppB.tile([128, 256], F32, tag="B")
                for j in range(NU):
                    nc.tensor.matmul(
                        ctx_ps[:], expT[:, j * 128:(j + 1) * 128],
                        win[:, j, 256:512], start=(j == 0), stop=(j == NU - 1))
                cslice = ctx_all[:, m * 256:(m + 1) * 256]
                nc.vector.scalar_tensor_tensor(
                    cslice, ctx_ps[:], recip[:], repf[:, 0:256],
                    ALU.mult, ALU.add)

                # LN2 stats + apply
                bns = wp.tile([128, 6], F32, tag="bns")
                nc.vector.bn_stats(bns[:], cslice)
                bna = wp.tile([128, 2], F32, tag="bna")
                nc.vector.bn_aggr(bna[:], bns[:])
                ve2 = wp.tile([128, 1], F32, tag="ve2")
                nc.vector.tensor_scalar(ve2[:], bna[:, 1:2], 1e-5, None,
                                        ALU.add)
                rstd2 = wp.tile([128, 1], F32, tag="rstd2")
                _newton(nc, wp, ve2[:], rstd2[:], "n2")
                ln2 = wp.tile([128, 256], BF, tag="ln2")
                nc.vector.tensor_scalar(ln2[:], cslice, bna[:, 0:1],
                                        rstd2[:, 0:1], ALU.subtract, ALU.mult)
                tpD = ppT.tile([128, SL], BF, tag="Tp")
                for ich in range(2):
                    nc.tensor.transpose(
                        tpD[:, ich * 128:(ich + 1) * 128],
                        ln2[:, ich * 128:(ich + 1) * 128], ident[:])
                lnT2 = wp.tile([128, 256], BF, tag="lnT2")
                nc.scalar.activation(lnT2[:], tpD[:, 0:256], AF.Copy)
                z_ps = ppB.tile([128, 256], F32, tag="B")
                for jch in range(2):
                    for hch in range(2):
                        nc.tensor.matmul(
                            z_ps[:, jch * 128:(jch + 1) * 128],
                            cw1[:, (hch * 2 + jch) * 128:(hch * 2 + jch + 1) * 128],
                            lnT2[:, hch * 128:(hch + 1) * 128],
                            start=(hch == 0), stop=(hch == 1))
                nc.scalar.activation(z_v[:, 0, msl], z_ps[:, 0:128], AF.Copy)
                nc.scalar.activation(z_v[:, 1, msl], z_ps[:, 128:256], AF.Copy)

            # ================= PHASE A2 (silu table) =================
            for m in range(TPC):
                msl = slice(m * 128, (m + 1) * 128)
                h1 = wp.tile([128, 256], BF, tag="h1")
                for jch in range(2):
                    nc.scalar.activation(
                        h1[:, jch * 128:(jch + 1) * 128], z_v[:, jch, msl],
                        AF.Silu, bias=ppb[:, 2 + jch:3 + jch])
                mlp_ps = ppB.tile([128, 256], F32, tag="B")
                for jch in range(2):
                    nc.tensor.matmul(
                        mlp_ps[:], h1[:, jch * 128:(jch + 1) * 128],
                        cw2[:, jch * 256:(jch + 1) * 256],
                        start=(jch == 0), stop=(jch == 1))
                ctx3 = wp.tile([128, 256], BF, tag="ctx3")
                nc.vector.tensor_tensor(
                    ctx3[:], mlp_ps[:], ctx_all[:, m * 256:(m + 1) * 256],
                    ALU.add)
                tpE = ppT.tile([128, SL], BF, tag="Tp")
                for ich in range(2):
                    nc.tensor.transpose(
                        tpE[:, ich * 128:(ich + 1) * 128],
                        ctx3[:, ich * 128:(ich + 1) * 128], ident[:])
                ctx3T = wp.tile([128, 256], BF, tag="ctx3T")
                nc.scalar.activation(ctx3T[:], tpE[:, 0:256], AF.Copy)

                bpA = ppA.tile([128, 512], F32, tag="A")
                for hch in range(2):
                    nc.tensor.matmul(
                        bpA[:], ctx3T[:, hch * 128:(hch + 1) * 128],
                        bpw[:, hch * 768:hch * 768 + 512],
                        start=(hch == 0), stop=(hch == 1))
                bpB = ppB.tile([128, 256], F32, tag="B")
                for hch in range(2):
                    nc.tensor.matmul(
                        bpB[:], ctx3T[:, hch * 128:(hch + 1) * 128],
                        bpw[:, hch * 768 + 512:hch * 768 + 768],
                        start=(hch == 0), stop=(hch == 1))
                s3c = wp.tile([128, 3], F32, tag="s3c")
                scrq = wp.tile([128, 256], F32, tag="scrq")
                for comp in range(3):
                    bsrc = bpA[:, comp * 256:(comp + 1) * 256] if comp < 2 \
                        else bpB[:]
                    nc.vector.scalar_tensor_tensor(
                        scrq[:], bsrc, 1.0,
                        tb_all[:, m * 768 + comp * 256:m * 768 + (comp + 1) * 256],
                        ALU.mult, ALU.mult,
                        accum_out=s3c[:, comp:comp + 1])
                scr3b = wp.tile([128, 3], F32, tag="scr3b")
                acc1 = wp.tile([128, 1], F32, tag="acc1")
                nc.vector.scalar_tensor_tensor(
                    scr3b[:], s3c[:], 1.0, csoh_all[:, m * 3:(m + 1) * 3],
                    ALU.mult, ALU.mult, accum_out=acc1[:])
                nc.vector.tensor_tensor(
                    outbuf[:, m:m + 1], acc1[:], outb_all[:, m:m + 1],
                    ALU.add)

            nc.sync.dma_start(out_d[:], outbuf[:])

    _bass_rust.move_matmul_waits_to_ldweights(nc.m)
    _bass_rust.generate_event_semaphores(nc)
    return nc


def _newton(nc, wp, ve_ap, out_ap, tag):
    """out = 1/sqrt(ve) via fast inverse sqrt + one Newton step."""
    t0 = wp.tile([128, 1], I32, tag=tag + "_t0")
    nc.vector.tensor_scalar(t0[:], ve_ap.bitcast(I32), 1, None,
                            ALU.arith_shift_right)
    y0i = wp.tile([128, 1], I32, tag=tag + "_y0")
    nc.vector.tensor_scalar(y0i[:], t0[:], 0x5F3759DF, -1,
                            ALU.subtract, ALU.mult)
    y0 = y0i[:].bitcast(F32)
    t1 = wp.tile([128, 1], F32, tag=tag + "_t1")
    nc.vector.tensor_tensor(t1[:], y0, y0, ALU.mult)
    t2 = wp.tile([128, 1], F32, tag=tag + "_t2")
    nc.vector.tensor_scalar(t2[:], t1[:], 0.5, None, ALU.mult)
    nc.vector.tensor_tensor(t2[:], t2[:], ve_ap, ALU.mult)
    t3 = wp.tile([128, 1], F32, tag=tag + "_t3")
    nc.vector.tensor_scalar(t3[:], t2[:], 1.5, -1.0, ALU.subtract, ALU.mult)
    nc.vector.tensor_tensor(out_ap, y0, t3[:], ALU.mult)


def _prepare(inputs):
    ins = {k: np.asarray(v) for k, v in inputs.items()}
    t_q = ins["t_q"].astype(np.float32)
    st = ins["sensor_time"].astype(np.float32)
    xy = ins["xy"].astype(np.float32)
    c = ins["c"].astype(np.float32)
    h = ins["h_states"].astype(np.float32)

    for G in (16, 24, 32):
        cores, B, TPC, idx = _pack(t_q, st, G)
        if TPC <= 8:
            break
    assert TPC <= 8, "packing exceeded 8 tiles/core (values_load register limit)"
    B64 = B * 64
    SL = G * 64
    Q = TPC * 128
    dtq = np.maximum(t_q - st[idx], 0.0).astype(np.float32)

    # ---- host-side parameter folds ----
    W_k = ins["btok_w"] @ ins["bk_w"]
    W_v = ins["btok_w"] @ ins["bv_w"]
    cv = ins["btok_b"] @ ins["bv_w"] + ins["bv_b"]
    Wq_s = (ins["bn_g"][:, None] * ins["bq_w"]) / 16.0
    bqb_s = (ins["bn_b"] @ ins["bq_w"] + ins["bq_b"]) / 16.0
    cw1_eff = ins["cln_g"][:, None] * ins["cw1"]
    cb1_eff = ins["cln_b"] @ ins["cw1"] + ins["cb1"]
    bp_b_eff = ins["cb2"] @ ins["bp_w"] + ins["bp_b"]
    temp = float(np.exp(ins["log_temp"][0]))
    u = Wq_s.sum(axis=0)
    kvec = (W_k @ u).astype(np.float32)
    kbvec = (W_k @ bqb_s).astype(np.float32)

    def chunk2(w):  # [256, X] -> [128, 2*X]
        x = w.shape[1]
        return np.ascontiguousarray(
            w.reshape(2, 128, x).transpose(1, 0, 2).reshape(128, 2 * x)
        ).astype(BF16)

    def chunk22(w):  # [256, 256] -> [128, 512]
        return np.ascontiguousarray(
            w.reshape(2, 128, 2, 128).transpose(1, 0, 2, 3).reshape(128, 512)
        ).astype(BF16)

    harm = np.arange(1, FH + 1, dtype=np.float32)
    colc_h = np.zeros((32, 4), np.float32)
    colc_h[:, 0] = np.tile(harm, 4)
    colc_h[:, 1] = np.concatenate(
        [np.zeros(8), np.full(8, 0.25), np.zeros(8), np.full(8, 0.25)])
    colc_h[:, 2] = ins["time_proj_w"][0]
    colc_h[:, 3] = ins["time_proj_b"]
    ppb_h = np.stack([
        ins["trunk_in_b"][0:128], ins["trunk_in_b"][128:256],
        cb1_eff[0:128], cb1_eff[128:256],
    ], axis=1).astype(np.float32)
    repbf_h = np.broadcast_to(
        np.concatenate([ins["to_b"], bp_b_eff]).astype(BF16), (128, 1536))
    repf_h = np.concatenate([
        np.broadcast_to(cv.astype(np.float32), (128, 256)),
        np.broadcast_to((temp * ins["comp_scale"]).astype(np.float32), (128, 3)),
        np.broadcast_to(ins["comp_bias"].astype(np.float32), (128, 3)),
    ], axis=1)
    expander_h = np.full((G, G * 64), NEG, np.float32)
    for s in range(G):
        expander_h[s, s * 64:(s + 1) * 64] = 0.0
    assert G <= 16

    shared = dict(
        wk=chunk22(W_k), wv=chunk2(W_v), trunkw=ins["trunk_in_w"].astype(BF16),
        bqw=chunk22(Wq_s), cw1w=chunk22(cw1_eff), cw2w=chunk2(ins["cw2"]),
        tow=chunk2(ins["to_w"]), bpw=chunk2(ins["bp_w"]),
        repbf=np.ascontiguousarray(repbf_h),
        repf=np.ascontiguousarray(repf_h),
        colc=colc_h, emb3=ins["comp_emb"].astype(BF16),
        iotag=np.arange(G, dtype=np.float32).reshape(G, 1),
        ppb=ppb_h, ident=np.eye(128, dtype=BF16),
        idf=np.eye(128, dtype=np.float32),
        onesb=np.ones((1, 128), BF16), onescol=np.ones((128, 1), BF16),
        sel4=np.repeat(np.eye(2, dtype=np.float32), 16, axis=1)
            .repeat(2, axis=0).astype(BF16),
        ones2=np.ones((2, 32), BF16),
    )

    in_maps = []
    slotmaps = []
    for lo, tiles in cores:
        hb = np.zeros((B, K, D), np.float32)
        nb = min(B, T - lo)
        hb[:nb] = h[lo:lo + nb]
        ht_h = np.ascontiguousarray(
            hb.reshape(B64, D).T.reshape(2, 128, B64).transpose(1, 0, 2)
            .reshape(128, 2 * B64)).astype(BF16)
        hb2 = hb.reshape(B64, D)
        kukb_rows = np.stack([-(hb2 @ kvec), hb2 @ kbvec])  # [2, B64]
        combt_h = np.zeros((34, TPC * SL), np.float32)
        xyhl_h = np.zeros((4, Q), np.float32)
        dthl_h = np.zeros((2, Q), np.float32)
        qrowb_h = np.zeros((2, Q), np.float32)
        moff_h = np.zeros((1, TPC), np.int32)
        smap = np.full((TPC, 128), -1, np.int64)
        for mth, (s, qsel, g, nreal) in enumerate(tiles):
            sl = slice(mth * 128, (mth + 1) * 128)
            dth = dtq[qsel].astype(BF16)
            dthl_h[0, sl] = dth.astype(np.float32)
            dthl_h[1, sl] = dtq[qsel] - dth.astype(np.float32)
            xh = xy[qsel, 0].astype(BF16)
            xyhl_h[0, sl] = xh.astype(np.float32)
            xyhl_h[1, sl] = xy[qsel, 0] - xh.astype(np.float32)
            yh = xy[qsel, 1].astype(BF16)
            xyhl_h[2, sl] = yh.astype(np.float32)
            xyhl_h[3, sl] = xy[qsel, 1] - yh.astype(np.float32)
            qrowb_h[0, sl] = c[qsel]
            qrowb_h[1, sl] = g.astype(np.float32)
            moff_h[0, mth] = 256 * s
            combt_h[0:G, mth * SL:(mth + 1) * SL] = expander_h
            combt_h[32:34, mth * SL:(mth + 1) * SL] = \
                kukb_rows[:, 64 * s:64 * s + SL]
            smap[mth, :nreal] = qsel[:nreal]
        in_maps.append(dict(ht=ht_h, combt=combt_h.astype(BF16),
                            xyhl=xyhl_h.astype(BF16), dthl=dthl_h.astype(BF16),
                            qrowb=qrowb_h.astype(BF16), moff=moff_h, **shared))
        slotmaps.append(smap)
    return in_maps, slotmaps, B, TPC, G


_last_run = None


def kernel(**inputs):
    global _last_run
    in_maps, slotmaps, B, TPC, G = _prepare(inputs)
    nc = _build(B, TPC, G)
    _last_run = run_bass_kernel_spmd(nc, in_maps, list(range(NCORES)))
    results = _last_run.results

    out_full = np.zeros(N, np.float32)
    for ci in range(NCORES):
        o = np.asarray(results[ci]["out"])  # [128, TPC]
        sm = slotmaps[ci]                   # [TPC, 128]
        for mth in range(TPC):
            valid = sm[mth] >= 0
            out_full[sm[mth][valid]] = o[valid, mth]
    return out_full
